# revision 16
# baseline (speedup 1.0000x reference)
"""Local (sliding-window) self-attention Bass kernel for 8 TRN2 NeuronCores.

Problem: B=4, T=4096, C=512, H=8 heads, head_dim=64, window=15.
Sharding: 8 cores = batch(4) x seq-halves(2). Each core processes 2048 query
tokens of one batch element; its x chunk carries a 7-token halo on each side
(zero-padded at sequence edges, matching the reference's jnp.pad semantics).

Host marshalling: x arrives pre-masked and pre-transposed (feature-major,
bf16) and the weights pre-cast to bf16 and pre-split, so the device only
DMAs and runs matmuls.

Device dataflow (per core; all matmuls bf16 with fp32 PSUM):
  xT [128, 4*NKV] <- DMA (pre-masked on host)
  qT/kT = W-stationary matmuls + bias (feature-major, via ACT copy); queries
    are stored shifted by +32 columns with zeroed pads so every key chunk
    can score its full in-window query range with one matmul
  v_tok = xT-stationary matmuls + bias, token-major, packed [64 v | 1 ones]
          per head (the ones column makes AV emit the softmax denominator)
  per 128-key chunk m (keys [128m, 128m+128)):
    scoresT [k, q] per head: ONE matmul against queries 128m-32..128m+128
      (N=160) -- every in-band (q, k) pair lives in exactly one chunk.
      Heads are placed 2-per-PSUM-bank so each bank's operand partition
      base is uniform (even heads base 0, odd base 64).
    exp -> alpha bf16 [128, 8*160]; one flat DVE tensor_mul with a 0/1
      band (2x bf16 mode) zeroes out-of-band entries exactly
  per 128-query block i:
    AV token-major, per head: av[q, 65] = alpha_i[:, 32:160].T @ v_aug
      (start) + alpha_{i+1}[:, 0:32].T @ v_aug_{i+1} into rows 96:128
      (stop, tile_position=(0, 96)) -- the zeroed band makes the full-128
      contraction of the tail exact; den accumulates in col 64
    reciprocal(den) * query-mask -> one DVE normalize -> avn bf16
    4 PE transposes -> attnT -> DVE int32-bitcast copy -> aT
    proj: aT-stationary matmuls; out = (bproj*mask) + psum via one DVE op
  Cross-engine pipelining: iteration i emits scores(i), out(i-2), units,
  av(i-1) so the PE never waits on the ACT exp or DVE work of the same
  block; ~5us of warm-up matmuls on `ident` keep the HAM clock gate at 8/8
  while the input DMAs land.
"""

import math
from contextlib import ExitStack

import ml_dtypes
import numpy as np

import concourse.bacc as bacc
import concourse.bass as bass
import concourse.mybir as mybir
import concourse.tile as tile
from concourse import bass_utils

B, T, C, H, WIN = 4, 4096, 512, 8, 15
D = C // H            # 64
PAD = WIN // 2        # 7
NTOK = T // 2         # 2048 query tokens per core
NKV = 2112            # kv rows per core: 7 + 2048 + 7 = 2062, padded to 2112
STR = 2176            # padded per-co column stride of kT/qT
NB = NTOK // 128      # 16 query blocks
SCALE = math.log(WIN) / D
F32 = mybir.dt.float32
BF16 = mybir.dt.bfloat16
I32 = mybir.dt.int32
FP8 = mybir.dt.float8e4
QKS = 16.0            # host scales Wq/Wk/bq/bk by this (keeps fp8 normal);
                      # folded back via the exp scale
NWARM = 112           # PE warm-up matmuls during the input DMA window


def _sc_col(h: int) -> int:
    """Column of head h in the scores PSUM tile: 2 heads per 512-col bank,
    even heads in banks 0/2 (operand base 0), odd in banks 1/3 (base 64)."""
    return ((h % 2) + (h // 4) * 2) * 512 + ((h // 2) % 2) * 160


def _al_col(h: int) -> int:
    """Column of head h in the packed alpha tile [128, 8*160]."""
    return ((h % 2) + (h // 4) * 2) * 320 + ((h // 2) % 2) * 160


def _mask_consts() -> dict:
    """0/1 band pattern (k on partitions, q on free) used to zero
    out-of-band alpha entries on the DVE after the exp.

    Chunk-relative: query col q' (absolute q = 128m - 32 + q'), key row k'
    (absolute k = 128m + k'): in-band iff k in [q, q+14] iff
    q'-32 <= k' <= q'-18.  Tiled x8 heads to match the alpha layout.
    """
    k = np.arange(128)[:, None]
    q = np.arange(160)[None, :]
    a = np.where((k >= q - 32) & (k <= q - 18), 1.0, 0.0).astype(np.float32)
    return {"band160": np.ascontiguousarray(
        np.tile(a, (1, 8)).astype(ml_dtypes.bfloat16))}


def _identity() -> np.ndarray:
    return np.eye(128, dtype=ml_dtypes.bfloat16)


def build_program() -> bacc.Bacc:
    nc = bacc.Bacc("TRN2", target_bir_lowering=False, debug=False,
                   enable_asserts=False, num_devices=8)

    xtd = nc.dram_tensor("xt", [C, NKV], BF16, kind="ExternalInput").ap()
    maskd = nc.dram_tensor("mask", [NKV], F32, kind="ExternalInput").ap()
    xt8d = nc.dram_tensor("xt8", [C, NKV], FP8, kind="ExternalInput").ap()
    wqd = nc.dram_tensor("wq", [C, C], FP8, kind="ExternalInput").ap()
    bqd = nc.dram_tensor("bq", [C], F32, kind="ExternalInput").ap()
    wkd = nc.dram_tensor("wk", [C, C], FP8, kind="ExternalInput").ap()
    wvd = nc.dram_tensor("wv", [C, C], BF16, kind="ExternalInput").ap()
    bkvd = nc.dram_tensor("bkv", [2 * C], F32, kind="ExternalInput").ap()
    wpd = nc.dram_tensor("wproj", [C, C], BF16, kind="ExternalInput").ap()
    bpd = nc.dram_tensor("bproj", [C], F32, kind="ExternalInput").ap()
    band160d = nc.dram_tensor("band160", [128, 1280], BF16, kind="ExternalInput").ap()
    identd = nc.dram_tensor("ident", [128, 128], BF16, kind="ExternalInput").ap()
    outd = nc.dram_tensor("out", [NTOK, C], F32, kind="ExternalOutput").ap()

    with tile.TileContext(nc) as tc, ExitStack() as ctx:
        sb = ctx.enter_context(tc.tile_pool(name="sb", bufs=1))
        sb_a = ctx.enter_context(tc.tile_pool(name="sb_a", bufs=3))
        sb_o = ctx.enter_context(tc.tile_pool(name="sb_o", bufs=3))
        pp_sc = ctx.enter_context(tc.tile_pool(name="pp_sc", bufs=1, space="PSUM"))
        pp_tr = ctx.enter_context(tc.tile_pool(name="pp_tr", bufs=1, space="PSUM"))
        pp_pr = ctx.enter_context(tc.tile_pool(name="pp_pr", bufs=1, space="PSUM"))
        pp_av = ctx.enter_context(tc.tile_pool(name="pp_av", bufs=1, space="PSUM"))

        # ---- persistent SBUF tensors ----
        xT = sb.tile([128, 4 * NKV], BF16, tag="xT")     # col ci*NKV + t
        qT = sb.tile([128, 4 * STR], BF16, tag="qT")     # col co*STR + 32 + t
        kT = sb.tile([128, 4 * STR], BF16, tag="kT")     # col co*STR + t
        aT = sb.tile([128, 4 * NTOK], BF16, tag="aT")    # col ct*NTOK + q
        v_tok = [sb.tile([128, 520], BF16, tag=f"vtok{i}", name=f"vtok{i}")
                 for i in range(17)]                     # col h*65: [64 v | 1]
        band160 = sb.tile([128, 1280], BF16, tag="band160")
        ident = sb.tile([128, 128], BF16, tag="ident")
        x8 = sb.tile([128, 4 * NKV], FP8, tag="x8")      # col ci*NKV + t
        wq8 = sb.tile([128, 4 * C], FP8, tag="wq8")      # col ci*C + c
        wk8 = sb.tile([128, 4 * C], FP8, tag="wk8")      # col ci*C + c
        wv = [sb.tile([128, C], BF16, tag=f"wv{i}", name=f"wv{i}") for i in range(4)]
        wp = [sb.tile([128, C], BF16, tag=f"wp{i}", name=f"wp{i}") for i in range(4)]
        bq_t = sb.tile([128, 4], F32, tag="bq")       # per-partition q bias
        bk_t = sb.tile([128, 4], F32, tag="bk")       # per-partition k bias
        bvB = sb.tile([128, C], F32, tag="bvB")       # v bias bcast over partitions
        bpB = sb.tile([128, C], F32, tag="bpB")       # proj bias bcast
        mq = sb.tile([128, NB], F32, tag="mq")        # query-token mask, per block

        # ---- constants / weights / x in (DMA only; no staging casts) ----
        # Order matters: ident first (feeds the PE warm-up), then the x/wk/wq
        # chunks needed by the first compute units, then the rest.
        nc.sync.dma_start(ident[:], identd)
        for ci in range(4):
            nc.sync.dma_start(wk8[:, ci * C:(ci + 1) * C],
                              wkd[ci * 128:(ci + 1) * 128, :])
        for ci in range(4):
            nc.sync.dma_start(x8[:, ci * NKV:ci * NKV + NKV],
                              xt8d[ci * 128:(ci + 1) * 128, :])
        for ci in range(4):
            nc.sync.dma_start(wq8[:, ci * C:(ci + 1) * C],
                              wqd[ci * 128:(ci + 1) * 128, :])
        nc.sync.dma_start(bq_t[:], bqd.rearrange("(a b) -> b a", b=128))
        nc.sync.dma_start(bk_t[:], bkvd[0:C].rearrange("(a b) -> b a", b=128))
        for ci in range(4):
            nc.sync.dma_start(xT[:, ci * NKV:ci * NKV + 528],
                              xtd[ci * 128:(ci + 1) * 128, 0:528])
        for ci in range(4):
            nc.sync.dma_start(wv[ci][:], wvd[ci * 128:(ci + 1) * 128, :])
        nc.sync.dma_start(bvB[:], bkvd[C:2 * C][None, :].broadcast_to((128, C)))
        for ci in range(4):
            nc.sync.dma_start(xT[:, ci * NKV + 528:ci * NKV + 1056],
                              xtd[ci * 128:(ci + 1) * 128, 528:1056])
        for ci in range(4):
            nc.sync.dma_start(xT[:, ci * NKV + 1056:ci * NKV + 2112],
                              xtd[ci * 128:(ci + 1) * 128, 1056:2112])
        for ci in range(4):
            nc.sync.dma_start(wp[ci][:], wpd[ci * 128:(ci + 1) * 128, :])
        nc.sync.dma_start(bpB[:], bpd[None, :].broadcast_to((128, C)))
        nc.sync.dma_start(mq[:], maskd[PAD:PAD + NTOK].rearrange("(a b) -> b a", b=128))
        nc.sync.dma_start(band160[:], band160d)

        # zero the query/key column pads (query lead 0:32, tail 2080:2176;
        # key tail 2112:2176) so padded scores stay finite
        qv4 = qT.rearrange("p (co t) -> p co t", co=4)
        kv4 = kT.rearrange("p (co t) -> p co t", co=4)
        nc.gpsimd.memset(qv4[:, :, 0:32], 0.0)
        nc.gpsimd.memset(qv4[:, :, 2080:2176], 0.0)
        nc.gpsimd.memset(kv4[:, :, 2112:2176], 0.0)

        # ---- PE warm-up: ~5us of matmuls on `ident` while x streams in, so
        # the HAM clock gate is at 8/8 by the time real work starts ----
        warm = pp_pr.tile([128, 512], F32, tag="pr", name="warm")
        for w in range(NWARM):
            nc.tensor.matmul(warm[:, 0:128], ident[:], ident[:],
                             start=True, stop=True, skip_group_check=True)

        # Unit-phase PSUM tiles alternate pp_pr / pp_tr so a unit's matmuls
        # don't wait on the previous unit's ACT copy draining a single buffer.
        ucnt = [0]

        def unit_ps(nm):
            pool = pp_pr if ucnt[0] % 2 == 0 else pp_tr
            ucnt[0] += 1
            return pool.tile([128, 512], F32, tag="pr" if pool is pp_pr else "tr",
                             name=nm)

        KCH = [512, 512, 512, 512, 64]

        def emit_kT_co(ch, co):
            t0 = 512 * ch
            w = KCH[ch]
            ps = unit_ps(f"u{ch}_{co}k")
            for j in range(2):
                nc.tensor.matmul(
                    ps[:, 0:w],
                    wk8[:, 2 * j * C:(2 * j + 2) * C]
                       .rearrange("p (two c) -> p two c", two=2)
                       [:, :, co * 128:(co + 1) * 128],
                    x8[:, 2 * j * NKV:(2 * j + 2) * NKV]
                      .rearrange("p (two t) -> p two t", two=2)
                      [:, :, t0:t0 + w],
                    start=(j == 0), stop=(j == 1),
                    perf_mode=mybir.MatmulPerfMode.DoubleRow)
            nc.scalar.activation(kT[:, co * STR + t0:co * STR + t0 + w],
                                 ps[:, 0:w],
                                 mybir.ActivationFunctionType.Identity,
                                 bias=bk_t[:, co:co + 1])

        def emit_qT_co(ch, co):
            t0 = 512 * ch
            ps = unit_ps(f"u{ch}_{co}q")
            for j in range(2):
                nc.tensor.matmul(
                    ps[:, 0:512],
                    wq8[:, 2 * j * C:(2 * j + 2) * C]
                       .rearrange("p (two c) -> p two c", two=2)
                       [:, :, co * 128:(co + 1) * 128],
                    x8[:, 2 * j * NKV:(2 * j + 2) * NKV]
                      .rearrange("p (two t) -> p two t", two=2)
                      [:, :, PAD + t0:PAD + t0 + 512],
                    start=(j == 0), stop=(j == 1),
                    perf_mode=mybir.MatmulPerfMode.DoubleRow)
            nc.scalar.activation(qT[:, co * STR + 32 + t0:co * STR + 32 + t0 + 512],
                                 ps[:, 0:512],
                                 mybir.ActivationFunctionType.Identity,
                                 bias=bq_t[:, co:co + 1])

        def emit_v(t):
            r0, r1 = t * 128, min((t + 1) * 128, NKV)
            rows = r1 - r0
            ps = unit_ps(f"u{t}v")
            for ci in range(4):
                nc.tensor.matmul(
                    ps[:rows, 0:512], xT[:, ci * NKV + r0:ci * NKV + r1],
                    wv[ci][:], start=(ci == 0), stop=(ci == 3))
            vv = v_tok[t].rearrange("p (h y) -> p h y", h=8)
            nc.gpsimd.memset(vv[:, :, 64:65], 1.0)
            if rows < 128:
                # zero the tail rows: the final block's full-128-contract
                # AV tail reads them (against band-zeroed alpha)
                nc.gpsimd.memset(vv[rows:128, :, 0:64], 0.0)
            nc.vector.scalar_tensor_tensor(
                vv[:rows, :, 0:64],
                ps[:rows, 0:512].rearrange("p (h y) -> p h y", h=8),
                1.0,
                bvB.rearrange("p (h y) -> p h y", h=8)[:rows],
                op0=mybir.AluOpType.mult, op1=mybir.AluOpType.add)

        # ---- attention, software-pipelined across engines ----
        alpha_t = [None] * (NB + 1)
        avn_t = [None] * NB

        def stage_scores(m):
            """Scores of key chunk m against its full in-window query range
            (160 queries starting at 128m-32; qT is stored with a +32 column
            shift so that range is cols m*128..m*128+160)."""
            sc = pp_sc.tile([128, 2048], F32, tag="sc", name=f"sc{m}")
            for h in range(8):
                co, hr = h // 2, (h % 2) * 64
                nc.tensor.matmul(
                    sc[:, _sc_col(h):_sc_col(h) + 160],
                    kT[hr:hr + 64, co * STR + m * 128:co * STR + m * 128 + 128],
                    qT[hr:hr + 64, co * STR + m * 128:co * STR + m * 128 + 160],
                    start=True, stop=True, skip_group_check=True)
            alpha = sb_a.tile([128, 1280], BF16, tag="alpha", name=f"al{m}")
            nc.scalar.activation(
                alpha.rearrange("p (bk y) -> p bk y", bk=4),
                sc.rearrange("p (bk y) -> p bk y", bk=4)[:, :, 0:320],
                mybir.ActivationFunctionType.Exp, scale=SCALE / (QKS * QKS))
            # zero out-of-band entries exactly (band160 is 0/1; flat APs keep
            # the DVE in its 2x bf16 mode)
            nc.vector.tensor_mul(alpha[:], alpha[:], band160[:])
            alpha_t[m] = alpha

        def stage_scores_tail():
            """Chunk 16 (keys 2048..2176, zero-padded past 2112): only its
            first 64 query columns (queries 2016..2080) matter; later query
            columns are past the sequence and never read."""
            sc = pp_sc.tile([128, 2048], F32, tag="sc", name="sc16")
            for h in range(8):
                co, hr = h // 2, (h % 2) * 64
                nc.tensor.matmul(
                    sc[:, _sc_col(h):_sc_col(h) + 64],
                    kT[hr:hr + 64, co * STR + 2048:co * STR + 2176],
                    qT[hr:hr + 64, co * STR + 2048:co * STR + 2112],
                    start=True, stop=True, skip_group_check=True)
            alpha = sb_a.tile([128, 1280], BF16, tag="alpha", name="al16")
            scv = sc.rearrange("p (bk y) -> p bk y", bk=4)
            alv = alpha.rearrange("p (bk y) -> p bk y", bk=4)
            bdv = band160.rearrange("p (bk y) -> p bk y", bk=4)
            for y0 in (0, 160):
                nc.scalar.activation(alv[:, :, y0:y0 + 64],
                                     scv[:, :, y0:y0 + 64],
                                     mybir.ActivationFunctionType.Exp,
                                     scale=SCALE / (QKS * QKS))
                nc.vector.tensor_mul(alv[:, :, y0:y0 + 64],
                                     alv[:, :, y0:y0 + 64],
                                     bdv[:, :, y0:y0 + 64])
            alpha_t[NB] = alpha

        def stage_av(i):
            alpha, alpha2 = alpha_t[i], alpha_t[i + 1]
            av = pp_av.tile([128, 1024], F32, tag="av", name=f"av{i}")
            for h in range(8):
                c0 = (h // 4) * 512 + (h % 4) * 65
                nc.tensor.matmul(
                    av[:, c0:c0 + 65],
                    alpha[:, _al_col(h) + 32:_al_col(h) + 160],
                    v_tok[i][:, h * 65:h * 65 + 65],
                    start=True, stop=False, skip_group_check=True)
                nc.tensor.matmul(
                    av[96:128, c0:c0 + 65],
                    alpha2[:, _al_col(h):_al_col(h) + 32],
                    v_tok[i + 1][:, h * 65:h * 65 + 65],
                    start=False, stop=True, skip_group_check=True,
                    tile_position=(0, 96))
            avv = (av.rearrange("p (a c) -> p a c", a=2)[:, :, 0:260]
                     .rearrange("p a (h y) -> p a h y", h=4))
            rden = sb_o.tile([128, 8], F32, tag="rden", name=f"rd{i}")
            nc.vector.reciprocal(rden.rearrange("p (a h) -> p a h", a=2),
                                 avv[:, :, :, 64:65].squeeze(3))
            avn = sb_o.tile([128, 512], BF16, tag="avn", name=f"avn{i}")
            for a in range(2):
                nc.vector.scalar_tensor_tensor(
                    avn[:, a * 256:(a + 1) * 256]
                       .rearrange("p (h y) -> p h y", h=4),
                    avv[:, a:a + 1, :, 0:64].squeeze(1), mq[:, i:i + 1],
                    rden[:, a * 4:(a + 1) * 4].unsqueeze(2)
                        .broadcast_to((128, 4, 64)),
                    op0=mybir.AluOpType.mult, op1=mybir.AluOpType.mult)
            avn_t[i] = avn

        def stage_out(i):
            avn = avn_t[i]
            tr = pp_tr.tile([128, 512], BF16, tag="tr", name=f"tr{i}")
            for ct in range(4):
                nc.tensor.transpose(
                    tr[:, ct * 128:(ct + 1) * 128],
                    avn[:, ct * 128:(ct + 1) * 128],
                    ident[:])
            # bf16 copy as int32 halves the DVE element count
            nc.vector.tensor_copy(
                aT.rearrange("p (a c) -> p a c", a=4)[:, :, i * 128:(i + 1) * 128]
                  .bitcast(I32),
                tr.rearrange("p (a c) -> p a c", a=4).bitcast(I32))
            pr = pp_pr.tile([128, 512], F32, tag="pr", name=f"pr{i}")
            for ct in range(4):
                nc.tensor.matmul(
                    pr[:], aT[:, ct * NTOK + i * 128:ct * NTOK + (i + 1) * 128],
                    wp[ct][:], start=(ct == 0), stop=(ct == 3))
            ot = sb_o.tile([128, C], F32, tag="ot", name=f"ot{i}")
            nc.vector.scalar_tensor_tensor(
                ot[:], bpB[:], mq[:, i:i + 1], pr[:],
                op0=mybir.AluOpType.mult, op1=mybir.AluOpType.add)
            nc.sync.dma_start(outd[i * 128:(i + 1) * 128, :], ot[:])

        # ---- prologue: first chunk of kT/qT, v 0..2 ----
        for co in range(4):
            emit_kT_co(0, co)
        for co in range(4):
            emit_qT_co(0, co)
        for t in range(3):
            emit_v(t)

        # Units per group: the next kT/qT chunks and the v tiles upcoming
        # iterations need, interleaved between blocks (2-4 per block).
        # Iteration order: scores(i), out(i-2), units, av(i-1): av(i-1)
        # consumes alpha_i, whose exp+band runs while out(i-2) and the
        # units occupy the PE.
        for g in range(4):
            if g < 2:
                units = ([lambda ch=g + 1, co=co: emit_kT_co(ch, co)
                          for co in range(4)]
                         + [lambda ch=g + 1, co=co: emit_qT_co(ch, co)
                            for co in range(4)]
                         + [lambda t=t: emit_v(t)
                            for t in range(4 * g + 3, 4 * g + 7)])
            elif g == 2:
                units = ([lambda co=co: emit_kT_co(3, co) for co in range(4)]
                         + [lambda co=co: emit_qT_co(3, co) for co in range(4)]
                         + [lambda t=t: emit_v(t) for t in (11, 12)])
            else:
                units = ([lambda t=t: emit_v(t) for t in (13, 14)]
                         + [lambda co=co: emit_kT_co(4, co) for co in range(4)]
                         + [lambda t=t: emit_v(t) for t in (15, 16)])
            ui = 0
            for bi, i in enumerate(range(4 * g, 4 * g + 4)):
                take = (len(units) * (bi + 1)) // 4 - ui
                stage_scores(i)
                for u in units[ui:ui + take]:
                    u()
                ui += take
                if i >= 3:
                    stage_out(i - 3)
                if i == NB - 1:
                    stage_scores_tail()
                if i >= 1:
                    stage_av(i - 1)
        stage_av(NB - 1)
        stage_out(NB - 3)
        stage_out(NB - 2)
        stage_out(NB - 1)

    nc.compile()
    return nc


_CACHE: dict = {}


def _get_program() -> bacc.Bacc:
    if "nc" not in _CACHE:
        _CACHE["nc"] = build_program()
    return _CACHE["nc"]


def _core_inputs(x, mask, Wq, bq, Wkv, bkv, Wproj, bproj):
    """Host-side marshalling: mask, halo-slice, transpose, cast. Returns the
    per-core input maps."""
    consts = _mask_consts()
    f8 = ml_dtypes.float8_e4m3fn
    wq8 = np.ascontiguousarray((np.asarray(Wq, np.float32) * QKS).astype(f8))
    wkv = np.asarray(Wkv, np.float32)
    wk8 = np.ascontiguousarray((wkv[:, 0:C] * QKS).astype(f8))
    wv8 = np.ascontiguousarray(wkv[:, C:2 * C].astype(ml_dtypes.bfloat16))
    wp8 = np.ascontiguousarray(np.asarray(Wproj, np.float32).astype(ml_dtypes.bfloat16))
    bkv2 = np.asarray(bkv, np.float32).copy()
    bkv2[0:C] *= QKS
    shared = {
        "wq": wq8, "wk": wk8, "wv": wv8, "wproj": wp8,
        "bq": np.asarray(bq, np.float32) * QKS, "bkv": bkv2,
        "bproj": np.asarray(bproj, np.float32),
        "ident": np.ascontiguousarray(_identity()), **consts,
    }
    if not np.all(mask == 1.0):
        x = x * mask[:, :, None]
    in_maps = []
    for core in range(8):
        b, h = divmod(core, 2)
        s = h * NTOK
        xc = np.zeros((NKV, C), np.float32)
        mc = np.zeros((NKV,), np.float32)
        lo, hi = max(0, s - PAD), min(T, s + NTOK + PAD)
        xc[lo - (s - PAD):lo - (s - PAD) + hi - lo] = x[b, lo:hi]
        mc[lo - (s - PAD):lo - (s - PAD) + hi - lo] = mask[b, lo:hi]
        xct = xc.T
        in_maps.append({
            "xt": np.ascontiguousarray(xct.astype(ml_dtypes.bfloat16)),
            "xt8": np.ascontiguousarray(xct.astype(f8)),
            "mask": mc,
            **shared,
        })
    return in_maps


def kernel(x, mask, Wq, bq, Wkv, bkv, Wproj, bproj) -> np.ndarray:
    x = np.asarray(x, np.float32)
    mask = np.asarray(mask, np.float32)
    nc = _get_program()
    in_maps = _core_inputs(x, mask, Wq, bq, Wkv, bkv, Wproj, bproj)
    res = bass_utils.run_bass_kernel_spmd(nc, in_maps, core_ids=list(range(8)))
    out = np.empty((B, T, C), np.float32)
    for core in range(8):
        b, h = divmod(core, 2)
        out[b, h * NTOK:(h + 1) * NTOK] = res.results[core]["out"]
    return out


# revision 17
# speedup vs baseline: 1.0393x; 1.0393x over previous
"""Local (sliding-window) self-attention Bass kernel for 8 TRN2 NeuronCores.

Problem: B=4, T=4096, C=512, H=8 heads, head_dim=64, window=15.
Sharding: 8 cores = batch(4) x seq-halves(2). Each core processes 2048 query
tokens of one batch element; its x chunk carries a 7-token halo on each side
(zero-padded at sequence edges, matching the reference's jnp.pad semantics).

Host marshalling: x arrives pre-masked and pre-transposed (feature-major,
bf16) and the weights pre-cast to bf16 and pre-split, so the device only
DMAs and runs matmuls.

Device dataflow (per core; all matmuls bf16 with fp32 PSUM):
  xT [128, 4*NKV] <- DMA (pre-masked on host)
  qT/kT = W-stationary matmuls + bias (feature-major, via ACT copy); queries
    are stored shifted by +32 columns with zeroed pads so every key chunk
    can score its full in-window query range with one matmul
  v_tok = xT-stationary matmuls + bias, token-major, packed [64 v | 1 ones]
          per head (the ones column makes AV emit the softmax denominator)
  per 128-key chunk m (keys [128m, 128m+128)):
    scoresT [k, q] per head: ONE matmul against queries 128m-32..128m+128
      (N=160) -- every in-band (q, k) pair lives in exactly one chunk.
      Heads are placed 2-per-PSUM-bank so each bank's operand partition
      base is uniform (even heads base 0, odd base 64).
    exp -> alpha bf16 [128, 8*160]; one flat DVE tensor_mul with a 0/1
      band (2x bf16 mode) zeroes out-of-band entries exactly
  per 128-query block i:
    AV token-major, per head: av[q, 65] = alpha_i[:, 32:160].T @ v_aug
      (start) + alpha_{i+1}[:, 0:32].T @ v_aug_{i+1} into rows 96:128
      (stop, tile_position=(0, 96)) -- the zeroed band makes the full-128
      contraction of the tail exact; den accumulates in col 64
    reciprocal(den) * query-mask -> one DVE normalize -> avn bf16
    4 PE transposes -> attnT -> DVE int32-bitcast copy -> aT
    proj: aT-stationary matmuls; out = (bproj*mask) + psum via one DVE op
  Cross-engine pipelining: iteration i emits scores(i), out(i-2), units,
  av(i-1) so the PE never waits on the ACT exp or DVE work of the same
  block; ~5us of warm-up matmuls on `ident` keep the HAM clock gate at 8/8
  while the input DMAs land.
"""

import math
from contextlib import ExitStack

import ml_dtypes
import numpy as np

import concourse.bacc as bacc
import concourse.bass as bass
import concourse.mybir as mybir
import concourse.tile as tile
from concourse import bass_utils

B, T, C, H, WIN = 4, 4096, 512, 8, 15
D = C // H            # 64
PAD = WIN // 2        # 7
NTOK = T // 2         # 2048 query tokens per core
NKV = 2112            # kv rows per core: 7 + 2048 + 7 = 2062, padded to 2112
STR = 2176            # padded per-co column stride of kT/qT
NB = NTOK // 128      # 16 query blocks
SCALE = math.log(WIN) / D
F32 = mybir.dt.float32
BF16 = mybir.dt.bfloat16
I32 = mybir.dt.int32
FP8 = mybir.dt.float8e4
QKS = 16.0            # host scales Wq/Wk/bq/bk by this (keeps fp8 normal);
                      # folded back via the exp scale
NWARM = 112           # PE warm-up matmuls during the input DMA window


def _sc_col(h: int) -> int:
    """Column of head h in the scores PSUM tile: 2 heads per 512-col bank,
    even heads in banks 0/2 (operand base 0), odd in banks 1/3 (base 64)."""
    return ((h % 2) + (h // 4) * 2) * 512 + ((h // 2) % 2) * 160


def _al_col(h: int) -> int:
    """Column of head h in the packed alpha tile [128, 8*160]."""
    return ((h % 2) + (h // 4) * 2) * 320 + ((h // 2) % 2) * 160


def _mask_consts() -> dict:
    """0/1 band pattern (k on partitions, q on free) used to zero
    out-of-band alpha entries on the DVE after the exp.

    Chunk-relative: query col q' (absolute q = 128m - 32 + q'), key row k'
    (absolute k = 128m + k'): in-band iff k in [q, q+14] iff
    q'-32 <= k' <= q'-18.  Tiled x8 heads to match the alpha layout.
    """
    k = np.arange(128)[:, None]
    q = np.arange(160)[None, :]
    a = np.where((k >= q - 32) & (k <= q - 18), 1.0, 0.0).astype(np.float32)
    return {"band160": np.ascontiguousarray(
        np.tile(a, (1, 8)).astype(ml_dtypes.bfloat16))}


def _identity() -> np.ndarray:
    return np.eye(128, dtype=ml_dtypes.bfloat16)


def build_program() -> bacc.Bacc:
    nc = bacc.Bacc("TRN2", target_bir_lowering=False, debug=False,
                   enable_asserts=False, num_devices=8)

    xtd = nc.dram_tensor("xt", [C, NKV], BF16, kind="ExternalInput").ap()
    maskd = nc.dram_tensor("mask", [NKV], F32, kind="ExternalInput").ap()
    xt8d = nc.dram_tensor("xt8", [C, NKV], FP8, kind="ExternalInput").ap()
    wqd = nc.dram_tensor("wq", [C, C], FP8, kind="ExternalInput").ap()
    bqd = nc.dram_tensor("bq", [C], F32, kind="ExternalInput").ap()
    wkd = nc.dram_tensor("wk", [C, C], FP8, kind="ExternalInput").ap()
    wvd = nc.dram_tensor("wv", [C, C], BF16, kind="ExternalInput").ap()
    bkvd = nc.dram_tensor("bkv", [2 * C], F32, kind="ExternalInput").ap()
    wpd = nc.dram_tensor("wproj", [C, C], BF16, kind="ExternalInput").ap()
    bpd = nc.dram_tensor("bproj", [C], F32, kind="ExternalInput").ap()
    band160d = nc.dram_tensor("band160", [128, 1280], BF16, kind="ExternalInput").ap()
    identd = nc.dram_tensor("ident", [128, 128], BF16, kind="ExternalInput").ap()
    outd = nc.dram_tensor("out", [NTOK, C], F32, kind="ExternalOutput").ap()

    with tile.TileContext(nc) as tc, ExitStack() as ctx:
        sb = ctx.enter_context(tc.tile_pool(name="sb", bufs=1))
        sb_a = ctx.enter_context(tc.tile_pool(name="sb_a", bufs=4))
        sb_o = ctx.enter_context(tc.tile_pool(name="sb_o", bufs=3))
        pp_sc = ctx.enter_context(tc.tile_pool(name="pp_sc", bufs=1, space="PSUM"))
        pp_tr = ctx.enter_context(tc.tile_pool(name="pp_tr", bufs=1, space="PSUM"))
        pp_pr = ctx.enter_context(tc.tile_pool(name="pp_pr", bufs=1, space="PSUM"))
        pp_av = ctx.enter_context(tc.tile_pool(name="pp_av", bufs=1, space="PSUM"))

        # ---- persistent SBUF tensors ----
        xT = sb.tile([128, 4 * NKV], BF16, tag="xT")     # col ci*NKV + t
        qT = sb.tile([128, 4 * STR], BF16, tag="qT")     # col co*STR + 32 + t
        kT = sb.tile([128, 4 * STR], BF16, tag="kT")     # col co*STR + t
        aT = sb.tile([128, 4 * NTOK], BF16, tag="aT")    # col ct*NTOK + q
        v_tok = [sb.tile([128, 520], BF16, tag=f"vtok{i}", name=f"vtok{i}")
                 for i in range(17)]                     # col h*65: [64 v | 1]
        band160 = sb.tile([128, 1280], BF16, tag="band160")
        ident = sb.tile([128, 128], BF16, tag="ident")
        x8 = sb.tile([128, 4 * NKV], FP8, tag="x8")      # col ci*NKV + t
        wq8 = sb.tile([128, 4 * C], FP8, tag="wq8")      # col ci*C + c
        wk8 = sb.tile([128, 4 * C], FP8, tag="wk8")      # col ci*C + c
        wv = [sb.tile([128, C], BF16, tag=f"wv{i}", name=f"wv{i}") for i in range(4)]
        wp = [sb.tile([128, C], BF16, tag=f"wp{i}", name=f"wp{i}") for i in range(4)]
        bq_t = sb.tile([128, 4], F32, tag="bq")       # per-partition q bias
        bk_t = sb.tile([128, 4], F32, tag="bk")       # per-partition k bias
        bvB = sb.tile([128, C], F32, tag="bvB")       # v bias bcast over partitions
        bpB = sb.tile([128, C], F32, tag="bpB")       # proj bias bcast
        mq = sb.tile([128, NB], F32, tag="mq")        # query-token mask, per block

        # ---- constants / weights / x in (DMA only; no staging casts) ----
        # Order matters: ident first (feeds the PE warm-up), then the x/wk/wq
        # chunks needed by the first compute units, then the rest.
        nc.sync.dma_start(ident[:], identd)
        for ci in range(4):
            nc.sync.dma_start(wk8[:, ci * C:(ci + 1) * C],
                              wkd[ci * 128:(ci + 1) * 128, :])
        for ci in range(4):
            nc.sync.dma_start(x8[:, ci * NKV:ci * NKV + NKV],
                              xt8d[ci * 128:(ci + 1) * 128, :])
        for ci in range(4):
            nc.sync.dma_start(wq8[:, ci * C:(ci + 1) * C],
                              wqd[ci * 128:(ci + 1) * 128, :])
        nc.sync.dma_start(bq_t[:], bqd.rearrange("(a b) -> b a", b=128))
        nc.sync.dma_start(bk_t[:], bkvd[0:C].rearrange("(a b) -> b a", b=128))
        for ci in range(4):
            nc.sync.dma_start(xT[:, ci * NKV:ci * NKV + 528],
                              xtd[ci * 128:(ci + 1) * 128, 0:528])
        for ci in range(4):
            nc.sync.dma_start(wv[ci][:], wvd[ci * 128:(ci + 1) * 128, :])
        nc.sync.dma_start(bvB[:], bkvd[C:2 * C][None, :].broadcast_to((128, C)))
        for ci in range(4):
            nc.sync.dma_start(xT[:, ci * NKV + 528:ci * NKV + 1056],
                              xtd[ci * 128:(ci + 1) * 128, 528:1056])
        for ci in range(4):
            nc.sync.dma_start(xT[:, ci * NKV + 1056:ci * NKV + 2112],
                              xtd[ci * 128:(ci + 1) * 128, 1056:2112])
        for ci in range(4):
            nc.sync.dma_start(wp[ci][:], wpd[ci * 128:(ci + 1) * 128, :])
        nc.sync.dma_start(bpB[:], bpd[None, :].broadcast_to((128, C)))
        nc.sync.dma_start(mq[:], maskd[PAD:PAD + NTOK].rearrange("(a b) -> b a", b=128))
        nc.sync.dma_start(band160[:], band160d)

        # zero the query/key column pads (query lead 0:32, tail 2080:2176;
        # key tail 2112:2176) so padded scores stay finite
        qv4 = qT.rearrange("p (co t) -> p co t", co=4)
        kv4 = kT.rearrange("p (co t) -> p co t", co=4)
        nc.gpsimd.memset(qv4[:, :, 0:32], 0.0)
        nc.gpsimd.memset(qv4[:, :, 2080:2176], 0.0)
        nc.gpsimd.memset(kv4[:, :, 2112:2176], 0.0)

        # ---- PE warm-up: ~5us of matmuls on `ident` while x streams in, so
        # the HAM clock gate is at 8/8 by the time real work starts ----
        warm = pp_pr.tile([128, 512], F32, tag="pr", name="warm")
        for w in range(NWARM):
            nc.tensor.matmul(warm[:, 0:128], ident[:], ident[:],
                             start=True, stop=True, skip_group_check=True)

        # Unit-phase PSUM tiles alternate pp_pr / pp_tr so a unit's matmuls
        # don't wait on the previous unit's ACT copy draining a single buffer.
        ucnt = [0]

        def unit_ps(nm):
            pool = pp_pr if ucnt[0] % 2 == 0 else pp_tr
            ucnt[0] += 1
            return pool.tile([128, 512], F32, tag="pr" if pool is pp_pr else "tr",
                             name=nm)

        KCH = [512, 512, 512, 512, 64]

        def emit_kT_co(ch, co):
            t0 = 512 * ch
            w = KCH[ch]
            ps = unit_ps(f"u{ch}_{co}k")
            for j in range(2):
                nc.tensor.matmul(
                    ps[:, 0:w],
                    wk8[:, 2 * j * C:(2 * j + 2) * C]
                       .rearrange("p (two c) -> p two c", two=2)
                       [:, :, co * 128:(co + 1) * 128],
                    x8[:, 2 * j * NKV:(2 * j + 2) * NKV]
                      .rearrange("p (two t) -> p two t", two=2)
                      [:, :, t0:t0 + w],
                    start=(j == 0), stop=(j == 1),
                    perf_mode=mybir.MatmulPerfMode.DoubleRow)
            nc.scalar.activation(kT[:, co * STR + t0:co * STR + t0 + w],
                                 ps[:, 0:w],
                                 mybir.ActivationFunctionType.Identity,
                                 bias=bk_t[:, co:co + 1])

        def emit_qT_co(ch, co):
            t0 = 512 * ch
            ps = unit_ps(f"u{ch}_{co}q")
            for j in range(2):
                nc.tensor.matmul(
                    ps[:, 0:512],
                    wq8[:, 2 * j * C:(2 * j + 2) * C]
                       .rearrange("p (two c) -> p two c", two=2)
                       [:, :, co * 128:(co + 1) * 128],
                    x8[:, 2 * j * NKV:(2 * j + 2) * NKV]
                      .rearrange("p (two t) -> p two t", two=2)
                      [:, :, PAD + t0:PAD + t0 + 512],
                    start=(j == 0), stop=(j == 1),
                    perf_mode=mybir.MatmulPerfMode.DoubleRow)
            nc.scalar.activation(qT[:, co * STR + 32 + t0:co * STR + 32 + t0 + 512],
                                 ps[:, 0:512],
                                 mybir.ActivationFunctionType.Identity,
                                 bias=bq_t[:, co:co + 1])

        def emit_v(t):
            r0, r1 = t * 128, min((t + 1) * 128, NKV)
            rows = r1 - r0
            ps = unit_ps(f"u{t}v")
            for ci in range(4):
                nc.tensor.matmul(
                    ps[:rows, 0:512], xT[:, ci * NKV + r0:ci * NKV + r1],
                    wv[ci][:], start=(ci == 0), stop=(ci == 3))
            vv = v_tok[t].rearrange("p (h y) -> p h y", h=8)
            nc.gpsimd.memset(vv[:, :, 64:65], 1.0)
            if rows < 128:
                # zero the tail rows: the final block's full-128-contract
                # AV tail reads them (against band-zeroed alpha)
                nc.gpsimd.memset(vv[rows:128, :, 0:64], 0.0)
            nc.vector.scalar_tensor_tensor(
                vv[:rows, :, 0:64],
                ps[:rows, 0:512].rearrange("p (h y) -> p h y", h=8),
                1.0,
                bvB.rearrange("p (h y) -> p h y", h=8)[:rows],
                op0=mybir.AluOpType.mult, op1=mybir.AluOpType.add)

        # ---- attention, software-pipelined across engines ----
        alpha_t = [None] * (NB + 1)
        avn_t = [None] * NB

        def stage_scores(m):
            """Scores of key chunk m against its full in-window query range
            (160 queries starting at 128m-32; qT is stored with a +32 column
            shift so that range is cols m*128..m*128+160)."""
            sc = pp_sc.tile([128, 2048], F32, tag="sc", name=f"sc{m}")
            for h in range(8):
                co, hr = h // 2, (h % 2) * 64
                nc.tensor.matmul(
                    sc[:, _sc_col(h):_sc_col(h) + 160],
                    kT[hr:hr + 64, co * STR + m * 128:co * STR + m * 128 + 128],
                    qT[hr:hr + 64, co * STR + m * 128:co * STR + m * 128 + 160],
                    start=True, stop=True, skip_group_check=True)
            alpha = sb_a.tile([128, 1280], BF16, tag="alpha", name=f"al{m}")
            nc.scalar.activation(
                alpha.rearrange("p (bk y) -> p bk y", bk=4),
                sc.rearrange("p (bk y) -> p bk y", bk=4)[:, :, 0:320],
                mybir.ActivationFunctionType.Exp, scale=SCALE / (QKS * QKS))
            # zero out-of-band entries exactly (band160 is 0/1; flat APs keep
            # the DVE in its 2x bf16 mode)
            nc.vector.tensor_mul(alpha[:], alpha[:], band160[:])
            alpha_t[m] = alpha

        def stage_scores_tail():
            """Chunk 16 (keys 2048..2176, zero-padded past 2112): only its
            first 64 query columns (queries 2016..2080) matter; later query
            columns are past the sequence and never read."""
            sc = pp_sc.tile([128, 2048], F32, tag="sc", name="sc16")
            for h in range(8):
                co, hr = h // 2, (h % 2) * 64
                nc.tensor.matmul(
                    sc[:, _sc_col(h):_sc_col(h) + 64],
                    kT[hr:hr + 64, co * STR + 2048:co * STR + 2176],
                    qT[hr:hr + 64, co * STR + 2048:co * STR + 2112],
                    start=True, stop=True, skip_group_check=True)
            alpha = sb_a.tile([128, 1280], BF16, tag="alpha", name="al16")
            scv = sc.rearrange("p (bk y) -> p bk y", bk=4)
            alv = alpha.rearrange("p (bk y) -> p bk y", bk=4)
            bdv = band160.rearrange("p (bk y) -> p bk y", bk=4)
            for y0 in (0, 160):
                nc.scalar.activation(alv[:, :, y0:y0 + 64],
                                     scv[:, :, y0:y0 + 64],
                                     mybir.ActivationFunctionType.Exp,
                                     scale=SCALE / (QKS * QKS))
                nc.vector.tensor_mul(alv[:, :, y0:y0 + 64],
                                     alv[:, :, y0:y0 + 64],
                                     bdv[:, :, y0:y0 + 64])
            alpha_t[NB] = alpha

        def stage_av(i):
            alpha, alpha2 = alpha_t[i], alpha_t[i + 1]
            av = pp_av.tile([128, 1024], F32, tag="av", name=f"av{i}")
            for h in range(8):
                c0 = (h // 4) * 512 + (h % 4) * 65
                nc.tensor.matmul(
                    av[:, c0:c0 + 65],
                    alpha[:, _al_col(h) + 32:_al_col(h) + 160],
                    v_tok[i][:, h * 65:h * 65 + 65],
                    start=True, stop=False, skip_group_check=True)
                nc.tensor.matmul(
                    av[96:128, c0:c0 + 65],
                    alpha2[:, _al_col(h):_al_col(h) + 32],
                    v_tok[i + 1][:, h * 65:h * 65 + 65],
                    start=False, stop=True, skip_group_check=True,
                    tile_position=(0, 96))
            avv = (av.rearrange("p (a c) -> p a c", a=2)[:, :, 0:260]
                     .rearrange("p a (h y) -> p a h y", h=4))
            rden = sb_o.tile([128, 8], F32, tag="rden", name=f"rd{i}")
            nc.vector.reciprocal(rden.rearrange("p (a h) -> p a h", a=2),
                                 avv[:, :, :, 64:65].squeeze(3))
            avn = sb_o.tile([128, 512], BF16, tag="avn", name=f"avn{i}")
            for a in range(2):
                nc.vector.scalar_tensor_tensor(
                    avn[:, a * 256:(a + 1) * 256]
                       .rearrange("p (h y) -> p h y", h=4),
                    avv[:, a:a + 1, :, 0:64].squeeze(1), mq[:, i:i + 1],
                    rden[:, a * 4:(a + 1) * 4].unsqueeze(2)
                        .broadcast_to((128, 4, 64)),
                    op0=mybir.AluOpType.mult, op1=mybir.AluOpType.mult)
            avn_t[i] = avn

        def stage_out(i):
            avn = avn_t[i]
            tr = pp_tr.tile([128, 512], BF16, tag="tr", name=f"tr{i}")
            for ct in range(4):
                nc.tensor.transpose(
                    tr[:, ct * 128:(ct + 1) * 128],
                    avn[:, ct * 128:(ct + 1) * 128],
                    ident[:])
            # bf16 copy as int32 halves the DVE element count
            nc.vector.tensor_copy(
                aT.rearrange("p (a c) -> p a c", a=4)[:, :, i * 128:(i + 1) * 128]
                  .bitcast(I32),
                tr.rearrange("p (a c) -> p a c", a=4).bitcast(I32))
            pr = pp_pr.tile([128, 512], F32, tag="pr", name=f"pr{i}")
            for ct in range(4):
                nc.tensor.matmul(
                    pr[:], aT[:, ct * NTOK + i * 128:ct * NTOK + (i + 1) * 128],
                    wp[ct][:], start=(ct == 0), stop=(ct == 3))
            ot = sb_o.tile([128, C], F32, tag="ot", name=f"ot{i}")
            nc.vector.scalar_tensor_tensor(
                ot[:], bpB[:], mq[:, i:i + 1], pr[:],
                op0=mybir.AluOpType.mult, op1=mybir.AluOpType.add)
            nc.sync.dma_start(outd[i * 128:(i + 1) * 128, :], ot[:])

        # ---- prologue: first chunk of kT/qT, v 0..2 ----
        for co in range(4):
            emit_kT_co(0, co)
        for co in range(4):
            emit_qT_co(0, co)
        for t in range(3):
            emit_v(t)

        # Units per group: the next kT/qT chunks and the v tiles upcoming
        # iterations need, interleaved between blocks (2-4 per block).
        # Iteration order: scores(i), out(i-2), units, av(i-1): av(i-1)
        # consumes alpha_i, whose exp+band runs while out(i-2) and the
        # units occupy the PE.
        for g in range(4):
            if g < 2:
                units = ([lambda ch=g + 1, co=co: emit_kT_co(ch, co)
                          for co in range(4)]
                         + [lambda ch=g + 1, co=co: emit_qT_co(ch, co)
                            for co in range(4)]
                         + [lambda t=t: emit_v(t)
                            for t in range(4 * g + 3, 4 * g + 7)])
            elif g == 2:
                units = ([lambda co=co: emit_kT_co(3, co) for co in range(4)]
                         + [lambda co=co: emit_qT_co(3, co) for co in range(4)]
                         + [lambda t=t: emit_v(t) for t in (11, 12)])
            else:
                units = ([lambda t=t: emit_v(t) for t in (13, 14)]
                         + [lambda co=co: emit_kT_co(4, co) for co in range(4)]
                         + [lambda t=t: emit_v(t) for t in (15, 16)])
            ui = 0
            for bi, i in enumerate(range(4 * g, 4 * g + 4)):
                take = (len(units) * (bi + 1)) // 4 - ui
                stage_scores(i)
                for u in units[ui:ui + take]:
                    u()
                ui += take
                if i >= 3:
                    stage_out(i - 3)
                if i == NB - 1:
                    stage_scores_tail()
                if i >= 2:
                    stage_av(i - 2)
        stage_av(NB - 2)
        stage_av(NB - 1)
        stage_out(NB - 3)
        stage_out(NB - 2)
        stage_out(NB - 1)

    nc.compile()
    return nc


_CACHE: dict = {}


def _get_program() -> bacc.Bacc:
    if "nc" not in _CACHE:
        _CACHE["nc"] = build_program()
    return _CACHE["nc"]


def _core_inputs(x, mask, Wq, bq, Wkv, bkv, Wproj, bproj):
    """Host-side marshalling: mask, halo-slice, transpose, cast. Returns the
    per-core input maps."""
    consts = _mask_consts()
    f8 = ml_dtypes.float8_e4m3fn
    wq8 = np.ascontiguousarray((np.asarray(Wq, np.float32) * QKS).astype(f8))
    wkv = np.asarray(Wkv, np.float32)
    wk8 = np.ascontiguousarray((wkv[:, 0:C] * QKS).astype(f8))
    wv8 = np.ascontiguousarray(wkv[:, C:2 * C].astype(ml_dtypes.bfloat16))
    wp8 = np.ascontiguousarray(np.asarray(Wproj, np.float32).astype(ml_dtypes.bfloat16))
    bkv2 = np.asarray(bkv, np.float32).copy()
    bkv2[0:C] *= QKS
    shared = {
        "wq": wq8, "wk": wk8, "wv": wv8, "wproj": wp8,
        "bq": np.asarray(bq, np.float32) * QKS, "bkv": bkv2,
        "bproj": np.asarray(bproj, np.float32),
        "ident": np.ascontiguousarray(_identity()), **consts,
    }
    if not np.all(mask == 1.0):
        x = x * mask[:, :, None]
    in_maps = []
    for core in range(8):
        b, h = divmod(core, 2)
        s = h * NTOK
        xc = np.zeros((NKV, C), np.float32)
        mc = np.zeros((NKV,), np.float32)
        lo, hi = max(0, s - PAD), min(T, s + NTOK + PAD)
        xc[lo - (s - PAD):lo - (s - PAD) + hi - lo] = x[b, lo:hi]
        mc[lo - (s - PAD):lo - (s - PAD) + hi - lo] = mask[b, lo:hi]
        xct = xc.T
        in_maps.append({
            "xt": np.ascontiguousarray(xct.astype(ml_dtypes.bfloat16)),
            "xt8": np.ascontiguousarray(xct.astype(f8)),
            "mask": mc,
            **shared,
        })
    return in_maps


def kernel(x, mask, Wq, bq, Wkv, bkv, Wproj, bproj) -> np.ndarray:
    x = np.asarray(x, np.float32)
    mask = np.asarray(mask, np.float32)
    nc = _get_program()
    in_maps = _core_inputs(x, mask, Wq, bq, Wkv, bkv, Wproj, bproj)
    res = bass_utils.run_bass_kernel_spmd(nc, in_maps, core_ids=list(range(8)))
    out = np.empty((B, T, C), np.float32)
    for core in range(8):
        b, h = divmod(core, 2)
        out[b, h * NTOK:(h + 1) * NTOK] = res.results[core]["out"]
    return out
